# revision 38
# baseline (speedup 1.0000x reference)
"""Dynamic (MoE-routed) 3x3 conv kernel for Trainium2, 8 NeuronCores.

Problem: nn_DynamicConv_670014898566
  x         [32, 64, 128, 128] f32
  w_route   [4, 64] f32
  b_route   [4] f32
  w_experts [4, 64, 64, 3, 3] f32
  y = per-sample conv2d(x, sigmoid(mean(x,HW) @ w_route.T + b_route) @ w_experts, SAME)

Sharding: data-parallel over batch, 4 samples per core (2 pairs of 2).

Host-side prep (numpy, free): expert kernels pre-transposed to lhsT layout
weT[c, (e,tap,o)] and replicated to both partition halves; routing matrix with
bias row, partition masks and a stacked identity packed into one const tensor.
This removes all PE transposes / casts / partition-shift DMAs from the per-pair
critical path: the DVE mix (4 ops) directly produces the conv lhsT.

Per-core device program (Tile framework):
  - x pair DMA-cast to bf16 [128, 16384] via 128-partition quarter slices
    (sample A channels on partitions 0-63, B on 64-127); pair1's descriptors
    queue right behind pair0's on the same SWDGE ring (no serialization dep)
  - warmup matmuls (fp32, dead writes to a scratch PSUM bank) keep the PE HAM
    clock at 8/8 through the load phase so conv starts at full rate
  - routing: free-dim reduces (DVE+Scalar halves per quarter, as loads land)
    -> masked pooled columns -> stacked-identity gather matmul -> logits matmul
    (bias via 1.0 row) -> sigmoid -> mask broadcast matmul -> 4-op DVE mix
    reading rbc straight from PSUM, last op writes bf16 lhsT
  - conv: per (sample h, chunk-parity q) stream, 9 shifted bf16 matmuls
    accumulate into one PSUM region; kw/kh edges handled by narrowed column
    ranges + shifted PSUM writes (no padding/wrap)
  - 4-way PE tile parallelism: positions (64h, 64q); accumulation stays
    within-position
  - pair1's reduces/routing are emitted at tuned points inside pair0's conv
    loop so each engine reaches them just after their data lands
"""

import sys

sys.path.insert(0, "/opt/trn_rl_repo")

import numpy as np

B, C, H, W = 32, 64, 128, 128
E = 4
HW = H * W
N_CORES = 8
NS = B // N_CORES          # samples per core = 4
NPAIR = NS // 2            # pairs per core = 2
NCHUNK = H // 4            # 32 chunks of 4 output rows per sample
NT = NCHUNK // 2           # 16 chunk-pairs per sample pair
# full-coverage tap first (owns start=True so PSUM has_written covers the bank)
TAPS = [(1, 1), (0, 0), (0, 1), (0, 2), (1, 0), (1, 2), (2, 0), (2, 1), (2, 2)]
NSLICE = 8                 # x-load slices per pair (2048 cols each)

# const tensor column layout
CC_M2COL = 0    # [128, 2]  mask2cols: col s = 1 on partitions 64s..64s+63
CC_STACKI = 2   # [128, 64] stacked identity [I64; I64]
CC_ROUTE = 66   # [65, 4]   rows 0-63 w_route.T/HW, row 64 = b_route
CC_MASK2 = 70   # [2, 128]  mask2[s, p] = 1 iff p//64 == s
CC_N = 198

_CACHE = {}


def _build_nc():
    import concourse.bacc as bacc
    import concourse.mybir as mybir
    import concourse.tile as tile

    dt = mybir.dt
    f32 = dt.float32
    bf16 = dt.bfloat16

    nc = bacc.Bacc("TRN2", target_bir_lowering=False, debug=False, num_devices=N_CORES)

    x_d = nc.dram_tensor("x", [NS, C, H, W], f32, kind="ExternalInput")
    wet_d = nc.dram_tensor("weT", [128, E * C * 9], f32, kind="ExternalInput")
    consts_d = nc.dram_tensor("consts", [128, CC_N], f32, kind="ExternalInput")
    # y in "diagonal" stage layout so every store is one 128-partition DMA
    # with 16KB-contiguous per-partition runs; host numpy reassembles:
    # [pair, cls, s, c, g, t2, 4*W] where cls0: (b,hf)=(s,s), cls1: (1-s,s)
    y_d = nc.dram_tensor(
        "y", [NPAIR, 2, 2, C, NT // 2, 2, 4 * W], f32, kind="ExternalOutput"
    )

    # x as [(pair*2+h)*C + c, hw] so one DMA covers both samples of a pair
    x_flat128 = x_d.ap().rearrange("b c h w -> (b c) (h w)")
    y_g = y_d.ap().rearrange("pp cls s c g t2 x -> pp cls (s c) g t2 x")

    with tile.TileContext(nc) as tc:
        with (
            tc.tile_pool(name="const", bufs=1) as cpool,
            tc.tile_pool(name="xp", bufs=2) as xpool,
            tc.tile_pool(name="mix", bufs=2) as mpool,
            tc.tile_pool(name="wt", bufs=2) as wtpool,
            tc.tile_pool(name="small", bufs=2) as spool_s,
            tc.tile_pool(name="stage", bufs=4) as stpool,
            tc.tile_pool(name="cps", bufs=6, space="PSUM") as convps,
            tc.tile_pool(name="rps", bufs=1, space="PSUM") as rps,
            tc.tile_pool(name="wps", bufs=1, space="PSUM") as wps,
        ):
            # ---------------- loads first ----------------
            # pair0 then pair1 x loads on the SWDGE ring back-to-back; each
            # slice is a full 128-partition cast DMA (all 16 SDMA engines);
            # the bf16 warmup source tile is memset between the two pair gens
            SL = HW // NSLICE
            xb_t = [
                xpool.tile([128, HW], bf16, tag="xt", name=f"xb_p{p}")
                for p in range(NPAIR)
            ]
            for p in range(NPAIR):
                ctx = nc.named_scope(f"load_p{p}"); ctx.__enter__()
                for i in range(NSLICE):
                    nc.gpsimd.dma_start(
                        xb_t[p][:, i * SL : (i + 1) * SL],
                        x_flat128[128 * p : 128 * (p + 1), i * SL : (i + 1) * SL],
                    )
                ctx.__exit__(None, None, None)

            # junk bf16 tile + PSUM bank for the conv-clock pre-ramp burst
            junk = cpool.tile([128, 512], bf16)
            nc.gpsimd.memset(junk[:], 0.0)
            wtile = wps.tile([128, 512], f32, tag="warm")

            # constants (HWDGE queue, lands in a few us)
            consts_sb = cpool.tile([128, CC_N], f32)
            nc.sync.dma_start(consts_sb[:], consts_d.ap())
            we_sb = cpool.tile([128, E * C * 9], f32)
            nc.sync.dma_start(we_sb[:], wet_d.ap())

            mask2cols = consts_sb[:, CC_M2COL : CC_M2COL + 2]
            stackI = consts_sb[:, CC_STACKI : CC_STACKI + 64]
            route_full = consts_sb[0 : C + 1, CC_ROUTE : CC_ROUTE + E]
            mask2 = consts_sb[0:2, CC_MASK2 : CC_MASK2 + 128]

            # persistent pooled2 lhsT [65, 2]; bias row set once
            pooled2sb = cpool.tile([C + 1, 2], f32)
            nc.gpsimd.memset(pooled2sb[C : C + 1, :], 1.0)

            # preload the sigmoid table so it's off the critical path
            scr11 = spool_s.tile([1, 1], f32, tag="scr11")
            nc.scalar.activation(
                scr11[:], consts_sb[0:1, 0:1], mybir.ActivationFunctionType.Sigmoid
            )

            # scratch sink for Scalar-engine reduce (activation Copy+accum)
            act_scratch = cpool.tile([128, 2048], bf16)

            # NOTE on PE warmup: deliberately ABSENT. Conv-quality activity
            # (4-position, ~95% duty) holds the HAM at 8/8 indefinitely —
            # every measured K=4 penalty window was caused by mediocre-duty
            # warmup streams, and warmup queues delayed the routing chain by
            # 10-28us. Cold-starting conv costs only ~2-3us of ramp, once.

            # ---------------- routing helpers ----------------
            pooled_t = [
                spool_s.tile([128, 12], f32, tag="pooled", name=f"pooled_{p}")
                for p in range(NPAIR)
            ]

            def emit_reduce_slice(p, k, after=None):
                # one reduce per load slice: DVE takes even slices, Scalar
                # odd; the LAST slice is split across both engines so the
                # final reduce off the critical path is ~1.2us not 2.4us.
                # `after` pins the op behind a same-engine instruction so the
                # Tile scheduler cannot hoist pair1 reduces ahead of pair0's
                # routing chain / stage copies (it reorders engine queues).
                ops = []
                if k == NSLICE - 1:
                    ops.append(nc.vector.reduce_sum(
                        pooled_t[p][:, 8:9],
                        xb_t[p][:, k * SL : k * SL + SL // 2],
                        axis=mybir.AxisListType.X,
                    ))
                    ops.append(nc.scalar.activation(
                        act_scratch[:, 0 : SL // 2],
                        xb_t[p][:, k * SL + SL // 2 : (k + 1) * SL],
                        mybir.ActivationFunctionType.Copy,
                        accum_out=pooled_t[p][:, 4 + k // 2 : 5 + k // 2],
                    ))
                elif k % 2 == 0:
                    ops.append(nc.vector.reduce_sum(
                        pooled_t[p][:, k // 2 : k // 2 + 1],
                        xb_t[p][:, k * SL : (k + 1) * SL],
                        axis=mybir.AxisListType.X,
                    ))
                else:
                    ops.append(nc.scalar.activation(
                        act_scratch[:],
                        xb_t[p][:, k * SL : (k + 1) * SL],
                        mybir.ActivationFunctionType.Copy,
                        accum_out=pooled_t[p][:, 4 + k // 2 : 5 + k // 2],
                    ))
                for ins in ops:
                    if after is not None:
                        tile.add_dep_helper(
                            ins.ins, after.ins, sync=True,
                            reason="pin p1 prep into p0 conv schedule",
                        )
                return ops[-1]

            def emit_route_chain(p, pe_after=None):
                # pooled tail -> routing weights in lhsT layout, ~6 engine ops
                ctx = nc.named_scope(f"route_p{p}"); ctx.__enter__()
                pooled = pooled_t[p]
                nc.vector.reduce_sum(
                    pooled[:, 9:10], pooled[:, 0:9], axis=mybir.AxisListType.X
                )
                # P2[p, s] = pooled[p] masked to half s
                P2 = spool_s.tile([128, 2], f32, tag="P2", name=f"P2_{p}")
                nc.vector.tensor_scalar_mul(P2[:], mask2cols, pooled[:, 9:10])
                # gather both samples' pooled vectors onto partitions 0-63
                g_ps = rps.tile([C, 2], f32, tag="rps", name=f"g_{p}")
                g_mm = nc.tensor.matmul(g_ps[:], stackI[:], P2[:], start=True, stop=True)
                if pe_after is not None:
                    tile.add_dep_helper(
                        g_mm.ins, pe_after.ins, sync=True,
                        reason="pin p1 routing matmuls behind p0 conv",
                    )
                nc.scalar.copy(pooled2sb[0:C, :], g_ps[:])
                # logits.T [s, e] incl. bias row, sigmoid -> routing
                l_ps = rps.tile([2, E], f32, tag="rps", name=f"l_{p}")
                nc.tensor.matmul(l_ps[:], pooled2sb[:], route_full, start=True, stop=True)
                rT = spool_s.tile([2, E], f32, tag="rT", name=f"rT_{p}")
                nc.scalar.activation(
                    rT[:], l_ps[:], mybir.ActivationFunctionType.Sigmoid
                )
                # broadcast routing over partitions: rbc[p, e] = r[s(p), e]
                rbc_ps = rps.tile([128, E], f32, tag="rps", name=f"rb_{p}")
                rbc_mm = nc.tensor.matmul(rbc_ps[:], mask2, rT[:], start=True, stop=True)
                if p == 0:
                    # conv-clock pre-ramp: a short 2-position burst (pinned
                    # after rbc so the scheduler can't hoist it into the idle
                    # load phase) trips the HAM to 8/8 during the mix, so the
                    # first conv matmuls run at 2.4 GHz
                    prev = rbc_mm
                    for wi in range(14):
                        hh = 64 * (wi % 2)
                        w_mm = nc.tensor.matmul(
                            wtile[hh : hh + 64, :],
                            junk[hh : hh + 64, 0:64],
                            junk[hh : hh + 64, :],
                            start=True, stop=True,
                        )
                        tile.add_dep_helper(
                            w_mm.ins, prev.ins, sync=True,
                            reason="pre-ramp burst stays after rbc",
                        )
                        prev = w_mm
                # mix expert kernels directly in lhsT layout:
                # wmixT[p, tap*64+o] = sum_e rbc[p, e] * weT[p, e*576 + tap*64 + o]
                mixa = mpool.tile([128, C * 9], f32, tag="mixa", name=f"mixa_{p}")
                mixb = mpool.tile([128, C * 9], f32, tag="mixb", name=f"mixb_{p}")
                wmixT = wtpool.tile([128, C * 9], bf16, tag="wmixT", name=f"wmixT_{p}")
                nc.vector.tensor_scalar_mul(mixa[:], we_sb[:, 0:576], rbc_ps[:, 0:1])
                nc.vector.scalar_tensor_tensor(
                    mixb[:], we_sb[:, 576:1152], rbc_ps[:, 1:2], mixa[:],
                    op0=mybir.AluOpType.mult, op1=mybir.AluOpType.add,
                )
                nc.vector.scalar_tensor_tensor(
                    mixa[:], we_sb[:, 1152:1728], rbc_ps[:, 2:3], mixb[:],
                    op0=mybir.AluOpType.mult, op1=mybir.AluOpType.add,
                )
                nc.vector.scalar_tensor_tensor(
                    wmixT[:], we_sb[:, 1728:2304], rbc_ps[:, 3:4], mixa[:],
                    op0=mybir.AluOpType.mult, op1=mybir.AluOpType.add,
                )
                ctx.__exit__(None, None, None)
                return wmixT

            # pair0 reduces consume slices as they land
            for k in range(NSLICE):
                emit_reduce_slice(0, k)
            wmixT_t = [emit_route_chain(0), None]

            # pair1 work is pinned into pair0's conv at these group marks so
            # each engine reaches it just after its data lands (deps anchor
            # it behind the group's stage copies / a conv matmul — emission
            # order alone is ignored by the scheduler)
            def p1_hook(g, cpA, cpB, mm):
                if g <= 3:
                    emit_reduce_slice(1, 2 * g, after=cpB)
                    emit_reduce_slice(1, 2 * g + 1, after=cpA)
                elif g == 5:
                    wmixT_t[1] = emit_route_chain(1, pe_after=mm)

            # ---------------- conv ----------------
            for p in range(NPAIR):
                conv_scope = nc.named_scope(f"conv_p{p}"); conv_scope.__enter__()
                xb = xb_t[p]
                wmixT = wmixT_t[p]
                xb3 = xb.rearrange("p (r c) -> p r c", c=W)
                for g in range(NT // 2):
                    # 2-group store batches except the last pair's second
                    # half (kept fine-grained to shrink the kernel tail)
                    fine = p == NPAIR - 1 and g >= NT // 2 - 4
                    if fine:
                        stA = stpool.tile([128, 1024], f32, tag="stage2", name=f"stA_{p}_{g}", bufs=4)
                        stB = stpool.tile([128, 1024], f32, tag="stage2", name=f"stB_{p}_{g}", bufs=4)
                        co = 0
                    elif g % 2 == 0:
                        stA = stpool.tile([128, 2048], f32, tag="stage", name=f"stA_{p}_{g}")
                        stB = stpool.tile([128, 2048], f32, tag="stage", name=f"stB_{p}_{g}")
                        co = 0
                    else:
                        co = 1024
                    for tg in range(2):
                        t = 2 * g + tg
                        psA = convps.tile([128, 512], f32, tag="cps", name=f"psA_{p}_{t}")
                        psB = convps.tile([128, 512], f32, tag="cps", name=f"psB_{p}_{t}")
                        psA3 = psA.rearrange("p (r c) -> p r c", c=W)
                        psB3 = psB.rearrange("p (r c) -> p r c", c=W)
                        # stream (h, q) -> psum region: (0,0)->psA[0:64],
                        # (1,1)->psA[64:128], (1,0)->psB[0:64], (0,1)->psB[64:128]
                        for tap_idx, (kh, kw) in enumerate(TAPS):
                            cstart = max(0, 1 - kw)
                            cend = min(W, W + 1 - kw)
                            ncols = cend - cstart
                            ic0 = cstart + kw - 1
                            for h in range(2):
                                for q in range(2):
                                    ps3 = psA3 if h == q else psB3
                                    j = 2 * t + q
                                    rstart = max(4 * j, 1 - kh)
                                    rend = min(4 * j + 4, H + 1 - kh)
                                    nrows = rend - rstart
                                    ir0 = rstart + kh - 1
                                    last_mm = nc.tensor.matmul(
                                        ps3[
                                            64 * q : 64 * q + 64,
                                            rstart - 4 * j : rstart - 4 * j + nrows,
                                            cstart:cend,
                                        ],
                                        wmixT[
                                            64 * h : 64 * h + 64,
                                            (3 * kh + kw) * 64 : (3 * kh + kw) * 64 + 64,
                                        ],
                                        xb3[
                                            64 * h : 64 * h + 64,
                                            ir0 : ir0 + nrows,
                                            ic0 : ic0 + ncols,
                                        ],
                                        start=(tap_idx == 0),
                                        stop=(tap_idx == len(TAPS) - 1),
                                    )
                        cpA = nc.scalar.copy(stA[:, co + tg * 512 : co + (tg + 1) * 512], psA[:])
                        cpB = nc.vector.tensor_copy(stB[:, co + tg * 512 : co + (tg + 1) * 512], psB[:])
                        if p == NPAIR - 1 and g == NT // 2 - 1:
                            # final group: store per chunk-pair so the first
                            # half's stores overlap the last matmuls and the
                            # kernel tail shrinks
                            sl = slice(tg * 512, (tg + 1) * 512)
                            nc.sync.dma_start(y_g[p, 0, :, g, tg, :], stA[:, sl])
                            nc.sync.dma_start(y_g[p, 1, :, g, tg, :], stB[:, sl])
                    if fine and g < NT // 2 - 1:
                        # single-group stores through the tail
                        stA4 = stA.rearrange("p (t2 x) -> p t2 x", t2=2)
                        stB4 = stB.rearrange("p (t2 x) -> p t2 x", t2=2)
                        nc.sync.dma_start(y_g[p, 0, :, g, :, :], stA4[:])
                        nc.sync.dma_start(y_g[p, 1, :, g, :, :], stB4[:])
                    elif not fine and g % 2 == 1:
                        # stage layout: stA = [A even chunks; B odd], stB = [B even; A odd]
                        stA4 = stA.rearrange("p (g2 t2 x) -> p g2 t2 x", g2=2, t2=2)
                        stB4 = stB.rearrange("p (g2 t2 x) -> p g2 t2 x", g2=2, t2=2)
                        gsl = slice(g - 1, g + 1)
                        nc.sync.dma_start(y_g[p, 0, :, gsl, :, :], stA4[:])
                        nc.sync.dma_start(y_g[p, 1, :, gsl, :, :], stB4[:])
                    if p == 0:
                        p1_hook(g, cpA, cpB, last_mm)
                conv_scope.__exit__(None, None, None)

    nc.compile()
    return nc


def _host_consts(inputs):
    w_route = np.ascontiguousarray(inputs["w_route"], dtype=np.float32)
    b_route = np.ascontiguousarray(inputs["b_route"], dtype=np.float32)
    w_experts = np.ascontiguousarray(inputs["w_experts"], dtype=np.float32)

    # weT[c, ((e*3+kh)*3+kw)*64 + o] = w_experts[e, o, c, kh, kw]
    wet = w_experts.transpose(2, 0, 3, 4, 1).reshape(C, E * C * 9)
    wet = np.ascontiguousarray(np.concatenate([wet, wet], axis=0))

    consts = np.zeros((128, CC_N), dtype=np.float32)
    consts[0:64, CC_M2COL] = 1.0
    consts[64:128, CC_M2COL + 1] = 1.0
    eye = np.eye(64, dtype=np.float32)
    consts[0:64, CC_STACKI : CC_STACKI + 64] = eye
    consts[64:128, CC_STACKI : CC_STACKI + 64] = eye
    consts[0:C, CC_ROUTE : CC_ROUTE + E] = w_route.T / HW
    consts[C, CC_ROUTE : CC_ROUTE + E] = b_route
    consts[0, CC_MASK2 : CC_MASK2 + 64] = 1.0
    consts[1, CC_MASK2 + 64 : CC_MASK2 + 128] = 1.0
    return wet, consts


def _get_nc():
    if "nc" not in _CACHE:
        _CACHE["nc"] = _build_nc()
    return _CACHE["nc"]


def _run(inputs, trace=False, **kw):
    from concourse import bass_utils

    nc = _get_nc()
    x = np.ascontiguousarray(inputs["x"], dtype=np.float32)
    wet, consts = _host_consts(inputs)
    in_maps = [
        {
            "x": x[i * NS : (i + 1) * NS],
            "weT": wet,
            "consts": consts,
        }
        for i in range(N_CORES)
    ]
    res = bass_utils.run_bass_kernel_spmd(
        nc, in_maps, core_ids=list(range(N_CORES)), trace=trace, **kw
    )
    # reassemble from the diagonal stage layout:
    # y_dev[pp, cls, s, c, g, t2, 4W]; cls0 -> (b,hf)=(s,s), cls1 -> (1-s,s)
    y = np.empty((B, C, H, W), dtype=np.float32)
    yb = y.reshape(N_CORES, NPAIR, 2, C, NT // 2, 2, 2, 4, W)  # b,c,g,t2,hf,r,w
    for i in range(N_CORES):
        yd = np.asarray(res.results[i]["y"]).reshape(
            NPAIR, 2, 2, C, NT // 2, 2, 4, W
        )
        for s in range(2):
            yb[i, :, s, :, :, :, s] = yd[:, 0, s]
            yb[i, :, 1 - s, :, :, :, s] = yd[:, 1, s]
    return y, res


def kernel(**inputs):
    y, _ = _run(inputs)
    return y


# revision 39
# speedup vs baseline: 1.1569x; 1.1569x over previous
"""Dynamic (MoE-routed) 3x3 conv kernel for Trainium2, 8 NeuronCores.

Problem: nn_DynamicConv_670014898566
  x         [32, 64, 128, 128] f32
  w_route   [4, 64] f32
  b_route   [4] f32
  w_experts [4, 64, 64, 3, 3] f32
  y = per-sample conv2d(x, sigmoid(mean(x,HW) @ w_route.T + b_route) @ w_experts, SAME)

Sharding: data-parallel over batch, 4 samples per core (2 pairs of 2).

Host-side prep (numpy, free): expert kernels pre-transposed to lhsT layout
weT[c, (e,tap,o)] and replicated to both partition halves; routing matrix with
bias row, partition masks and a stacked identity packed into one const tensor.
This removes all PE transposes / casts / partition-shift DMAs from the per-pair
critical path: the DVE mix (4 ops) directly produces the conv lhsT.

Per-core device program (Tile framework):
  - x pair DMA-cast to bf16 [128, 16384] via 128-partition quarter slices
    (sample A channels on partitions 0-63, B on 64-127); pair1's descriptors
    queue right behind pair0's on the same SWDGE ring (no serialization dep)
  - warmup matmuls (fp32, dead writes to a scratch PSUM bank) keep the PE HAM
    clock at 8/8 through the load phase so conv starts at full rate
  - routing: free-dim reduces (DVE+Scalar halves per quarter, as loads land)
    -> masked pooled columns -> stacked-identity gather matmul -> logits matmul
    (bias via 1.0 row) -> sigmoid -> mask broadcast matmul -> 4-op DVE mix
    reading rbc straight from PSUM, last op writes bf16 lhsT
  - conv: per (sample h, chunk-parity q) stream, 9 shifted bf16 matmuls
    accumulate into one PSUM region; kw/kh edges handled by narrowed column
    ranges + shifted PSUM writes (no padding/wrap)
  - 4-way PE tile parallelism: positions (64h, 64q); accumulation stays
    within-position
  - pair1's reduces/routing are emitted at tuned points inside pair0's conv
    loop so each engine reaches them just after their data lands
"""

import sys

sys.path.insert(0, "/opt/trn_rl_repo")

import numpy as np

B, C, H, W = 32, 64, 128, 128
E = 4
HW = H * W
N_CORES = 8
NS = B // N_CORES          # samples per core = 4
NPAIR = NS // 2            # pairs per core = 2
NCHUNK = H // 4            # 32 chunks of 4 output rows per sample
NT = NCHUNK // 2           # 16 chunk-pairs per sample pair
# full-coverage tap first (owns start=True so PSUM has_written covers the bank)
TAPS = [(1, 1), (0, 0), (0, 1), (0, 2), (1, 0), (1, 2), (2, 0), (2, 1), (2, 2)]
NSLICE = 8                 # x-load slices per pair (2048 cols each)

# const tensor column layout
CC_M2COL = 0    # [128, 2]  mask2cols: col s = 1 on partitions 64s..64s+63
CC_STACKI = 2   # [128, 64] stacked identity [I64; I64]
CC_ROUTE = 66   # [65, 4]   rows 0-63 w_route.T/HW, row 64 = b_route
CC_MASK2 = 70   # [2, 128]  mask2[s, p] = 1 iff p//64 == s
CC_N = 198

_CACHE = {}


def _build_nc():
    import concourse.bacc as bacc
    import concourse.mybir as mybir
    import concourse.tile as tile

    dt = mybir.dt
    f32 = dt.float32
    bf16 = dt.bfloat16

    nc = bacc.Bacc("TRN2", target_bir_lowering=False, debug=False, num_devices=N_CORES)

    x_d = nc.dram_tensor("x", [NS, C, H, W], f32, kind="ExternalInput")
    wet_d = nc.dram_tensor("weT", [128, E * C * 9], f32, kind="ExternalInput")
    consts_d = nc.dram_tensor("consts", [128, CC_N], f32, kind="ExternalInput")
    # y in "diagonal" stage layout so every store is one 128-partition DMA
    # with 16KB-contiguous per-partition runs; host numpy reassembles:
    # [pair, cls, s, c, g, t2, 4*W] where cls0: (b,hf)=(s,s), cls1: (1-s,s)
    y_d = nc.dram_tensor(
        "y", [NPAIR, 2, 2, C, NT // 2, 2, 4 * W], f32, kind="ExternalOutput"
    )

    # x as [(pair*2+h)*C + c, hw] so one DMA covers both samples of a pair
    x_flat128 = x_d.ap().rearrange("b c h w -> (b c) (h w)")
    y_g = y_d.ap().rearrange("pp cls s c g t2 x -> pp cls (s c) g t2 x")

    with tile.TileContext(nc) as tc:
        with (
            tc.tile_pool(name="const", bufs=1) as cpool,
            tc.tile_pool(name="xp", bufs=2) as xpool,
            tc.tile_pool(name="mix", bufs=2) as mpool,
            tc.tile_pool(name="wt", bufs=2) as wtpool,
            tc.tile_pool(name="small", bufs=2) as spool_s,
            tc.tile_pool(name="stage", bufs=4) as stpool,
            tc.tile_pool(name="cps", bufs=6, space="PSUM") as convps,
            tc.tile_pool(name="rps", bufs=1, space="PSUM") as rps,
            tc.tile_pool(name="wps", bufs=1, space="PSUM") as wps,
        ):
            # ---------------- loads first ----------------
            # pair0 then pair1 x loads on the SWDGE ring back-to-back; each
            # slice is a full 128-partition cast DMA (all 16 SDMA engines);
            # the bf16 warmup source tile is memset between the two pair gens
            SL = HW // NSLICE
            xb_t = [
                xpool.tile([128, HW], bf16, tag="xt", name=f"xb_p{p}")
                for p in range(NPAIR)
            ]
            for p in range(NPAIR):
                ctx = nc.named_scope(f"load_p{p}"); ctx.__enter__()
                for i in range(NSLICE):
                    nc.gpsimd.dma_start(
                        xb_t[p][:, i * SL : (i + 1) * SL],
                        x_flat128[128 * p : 128 * (p + 1), i * SL : (i + 1) * SL],
                    )
                ctx.__exit__(None, None, None)

            # junk bf16 tile + PSUM bank for the conv-clock pre-ramp burst
            junk = cpool.tile([128, 512], bf16)
            nc.gpsimd.memset(junk[:], 0.0)
            wtile = wps.tile([128, 512], f32, tag="warm")

            # constants (HWDGE queue, lands in a few us)
            consts_sb = cpool.tile([128, CC_N], f32)
            nc.sync.dma_start(consts_sb[:], consts_d.ap())
            we_sb = cpool.tile([128, E * C * 9], f32)
            nc.sync.dma_start(we_sb[:], wet_d.ap())

            mask2cols = consts_sb[:, CC_M2COL : CC_M2COL + 2]
            stackI = consts_sb[:, CC_STACKI : CC_STACKI + 64]
            route_full = consts_sb[0 : C + 1, CC_ROUTE : CC_ROUTE + E]
            mask2 = consts_sb[0:2, CC_MASK2 : CC_MASK2 + 128]

            # persistent pooled2 lhsT [65, 2]; bias row set once
            pooled2sb = cpool.tile([C + 1, 2], f32)
            nc.gpsimd.memset(pooled2sb[C : C + 1, :], 1.0)

            # preload the sigmoid table so it's off the critical path
            scr11 = spool_s.tile([1, 1], f32, tag="scr11")
            nc.scalar.activation(
                scr11[:], consts_sb[0:1, 0:1], mybir.ActivationFunctionType.Sigmoid
            )

            # scratch sink for Scalar-engine reduce (activation Copy+accum)
            act_scratch = cpool.tile([128, 2048], bf16)

            # NOTE on PE warmup: deliberately ABSENT. Conv-quality activity
            # (4-position, ~95% duty) holds the HAM at 8/8 indefinitely —
            # every measured K=4 penalty window was caused by mediocre-duty
            # warmup streams, and warmup queues delayed the routing chain by
            # 10-28us. Cold-starting conv costs only ~2-3us of ramp, once.

            # ---------------- routing helpers ----------------
            pooled_t = [
                spool_s.tile([128, 12], f32, tag="pooled", name=f"pooled_{p}")
                for p in range(NPAIR)
            ]

            def emit_reduce_slice(p, k, after=None):
                # one reduce per load slice: DVE takes even slices, Scalar
                # odd; the LAST slice is split across both engines so the
                # final reduce off the critical path is ~1.2us not 2.4us.
                # `after` pins the op behind a same-engine instruction so the
                # Tile scheduler cannot hoist pair1 reduces ahead of pair0's
                # routing chain / stage copies (it reorders engine queues).
                ops = []
                if k % 2 == 0:
                    ops.append(nc.vector.reduce_sum(
                        pooled_t[p][:, k // 2 : k // 2 + 1],
                        xb_t[p][:, k * SL : (k + 1) * SL],
                        axis=mybir.AxisListType.X,
                    ))
                else:
                    ops.append(nc.scalar.activation(
                        act_scratch[:],
                        xb_t[p][:, k * SL : (k + 1) * SL],
                        mybir.ActivationFunctionType.Copy,
                        accum_out=pooled_t[p][:, 4 + k // 2 : 5 + k // 2],
                    ))
                for ins in ops:
                    if after is not None:
                        tile.add_dep_helper(
                            ins.ins, after.ins, sync=True,
                            reason="pin p1 prep into p0 conv schedule",
                        )
                return ops[-1]

            def emit_route_chain(p, pe_after=None):
                # pooled tail -> routing weights in lhsT layout, ~6 engine ops
                ctx = nc.named_scope(f"route_p{p}"); ctx.__enter__()
                pooled = pooled_t[p]
                nc.vector.reduce_sum(
                    pooled[:, 8:9], pooled[:, 0:8], axis=mybir.AxisListType.X
                )
                # P2[p, s] = pooled[p] masked to half s
                P2 = spool_s.tile([128, 2], f32, tag="P2", name=f"P2_{p}")
                nc.vector.tensor_scalar_mul(P2[:], mask2cols, pooled[:, 8:9])
                # gather both samples' pooled vectors onto partitions 0-63
                g_ps = rps.tile([C, 2], f32, tag="rps", name=f"g_{p}")
                g_mm = nc.tensor.matmul(g_ps[:], stackI[:], P2[:], start=True, stop=True)
                if pe_after is not None:
                    tile.add_dep_helper(
                        g_mm.ins, pe_after.ins, sync=True,
                        reason="pin p1 routing matmuls behind p0 conv",
                    )
                nc.scalar.copy(pooled2sb[0:C, :], g_ps[:])
                # logits.T [s, e] incl. bias row, sigmoid -> routing
                l_ps = rps.tile([2, E], f32, tag="rps", name=f"l_{p}")
                nc.tensor.matmul(l_ps[:], pooled2sb[:], route_full, start=True, stop=True)
                rT = spool_s.tile([2, E], f32, tag="rT", name=f"rT_{p}")
                nc.scalar.activation(
                    rT[:], l_ps[:], mybir.ActivationFunctionType.Sigmoid
                )
                # broadcast routing over partitions: rbc[p, e] = r[s(p), e]
                rbc_ps = rps.tile([128, E], f32, tag="rps", name=f"rb_{p}")
                rbc_mm = nc.tensor.matmul(rbc_ps[:], mask2, rT[:], start=True, stop=True)
                if p == 0:
                    # conv-clock pre-ramp: a short 2-position burst (pinned
                    # after rbc so the scheduler can't hoist it into the idle
                    # load phase) trips the HAM to 8/8 during the mix, so the
                    # first conv matmuls run at 2.4 GHz
                    prev = rbc_mm
                    for wi in range(14):
                        hh = 64 * (wi % 2)
                        w_mm = nc.tensor.matmul(
                            wtile[hh : hh + 64, :],
                            junk[hh : hh + 64, 0:64],
                            junk[hh : hh + 64, :],
                            start=True, stop=True,
                        )
                        tile.add_dep_helper(
                            w_mm.ins, prev.ins, sync=True,
                            reason="pre-ramp burst stays after rbc",
                        )
                        prev = w_mm
                # mix expert kernels directly in lhsT layout:
                # wmixT[p, tap*64+o] = sum_e rbc[p, e] * weT[p, e*576 + tap*64 + o]
                mixa = mpool.tile([128, C * 9], f32, tag="mixa", name=f"mixa_{p}")
                mixb = mpool.tile([128, C * 9], f32, tag="mixb", name=f"mixb_{p}")
                wmixT = wtpool.tile([128, C * 9], bf16, tag="wmixT", name=f"wmixT_{p}")
                nc.vector.tensor_scalar_mul(mixa[:], we_sb[:, 0:576], rbc_ps[:, 0:1])
                nc.vector.scalar_tensor_tensor(
                    mixb[:], we_sb[:, 576:1152], rbc_ps[:, 1:2], mixa[:],
                    op0=mybir.AluOpType.mult, op1=mybir.AluOpType.add,
                )
                nc.vector.scalar_tensor_tensor(
                    mixa[:], we_sb[:, 1152:1728], rbc_ps[:, 2:3], mixb[:],
                    op0=mybir.AluOpType.mult, op1=mybir.AluOpType.add,
                )
                nc.vector.scalar_tensor_tensor(
                    wmixT[:], we_sb[:, 1728:2304], rbc_ps[:, 3:4], mixa[:],
                    op0=mybir.AluOpType.mult, op1=mybir.AluOpType.add,
                )
                ctx.__exit__(None, None, None)
                return wmixT

            # pair0 reduces consume slices as they land
            for k in range(NSLICE):
                emit_reduce_slice(0, k)
            wmixT_t = [emit_route_chain(0), None]

            # pair1 work is pinned into pair0's conv at these group marks so
            # each engine reaches it just after its data lands (deps anchor
            # it behind the group's stage copies / a conv matmul — emission
            # order alone is ignored by the scheduler)
            def p1_hook(g, cpA, cpB, mm):
                if g <= 3:
                    emit_reduce_slice(1, 2 * g, after=cpB)
                    emit_reduce_slice(1, 2 * g + 1, after=cpA)
                elif g == 5:
                    wmixT_t[1] = emit_route_chain(1, pe_after=mm)

            # ---------------- conv ----------------
            for p in range(NPAIR):
                conv_scope = nc.named_scope(f"conv_p{p}"); conv_scope.__enter__()
                xb = xb_t[p]
                wmixT = wmixT_t[p]
                xb3 = xb.rearrange("p (r c) -> p r c", c=W)
                for g in range(NT // 2):
                    # 2-group store batches except the last pair's second
                    # half (kept fine-grained to shrink the kernel tail)
                    fine = p == NPAIR - 1 and g >= NT // 2 - 4
                    if fine:
                        stA = stpool.tile([128, 1024], f32, tag="stage2", name=f"stA_{p}_{g}", bufs=4)
                        stB = stpool.tile([128, 1024], f32, tag="stage2", name=f"stB_{p}_{g}", bufs=4)
                        co = 0
                    elif g % 2 == 0:
                        stA = stpool.tile([128, 2048], f32, tag="stage", name=f"stA_{p}_{g}")
                        stB = stpool.tile([128, 2048], f32, tag="stage", name=f"stB_{p}_{g}")
                        co = 0
                    else:
                        co = 1024
                    for tg in range(2):
                        t = 2 * g + tg
                        psA = convps.tile([128, 512], f32, tag="cps", name=f"psA_{p}_{t}")
                        psB = convps.tile([128, 512], f32, tag="cps", name=f"psB_{p}_{t}")
                        psA3 = psA.rearrange("p (r c) -> p r c", c=W)
                        psB3 = psB.rearrange("p (r c) -> p r c", c=W)
                        # stream (h, q) -> psum region: (0,0)->psA[0:64],
                        # (1,1)->psA[64:128], (1,0)->psB[0:64], (0,1)->psB[64:128]
                        for tap_idx, (kh, kw) in enumerate(TAPS):
                            cstart = max(0, 1 - kw)
                            cend = min(W, W + 1 - kw)
                            ncols = cend - cstart
                            ic0 = cstart + kw - 1
                            for h in range(2):
                                for q in range(2):
                                    ps3 = psA3 if h == q else psB3
                                    j = 2 * t + q
                                    rstart = max(4 * j, 1 - kh)
                                    rend = min(4 * j + 4, H + 1 - kh)
                                    nrows = rend - rstart
                                    ir0 = rstart + kh - 1
                                    last_mm = nc.tensor.matmul(
                                        ps3[
                                            64 * q : 64 * q + 64,
                                            rstart - 4 * j : rstart - 4 * j + nrows,
                                            cstart:cend,
                                        ],
                                        wmixT[
                                            64 * h : 64 * h + 64,
                                            (3 * kh + kw) * 64 : (3 * kh + kw) * 64 + 64,
                                        ],
                                        xb3[
                                            64 * h : 64 * h + 64,
                                            ir0 : ir0 + nrows,
                                            ic0 : ic0 + ncols,
                                        ],
                                        start=(tap_idx == 0),
                                        stop=(tap_idx == len(TAPS) - 1),
                                    )
                        cpA = nc.scalar.copy(stA[:, co + tg * 512 : co + (tg + 1) * 512], psA[:])
                        cpB = nc.vector.tensor_copy(stB[:, co + tg * 512 : co + (tg + 1) * 512], psB[:])
                        if p == NPAIR - 1 and g == NT // 2 - 1:
                            # final group: store per chunk-pair so the first
                            # half's stores overlap the last matmuls and the
                            # kernel tail shrinks
                            sl = slice(tg * 512, (tg + 1) * 512)
                            nc.sync.dma_start(y_g[p, 0, :, g, tg, :], stA[:, sl])
                            nc.sync.dma_start(y_g[p, 1, :, g, tg, :], stB[:, sl])
                    if fine and g < NT // 2 - 1:
                        # single-group stores through the tail
                        stA4 = stA.rearrange("p (t2 x) -> p t2 x", t2=2)
                        stB4 = stB.rearrange("p (t2 x) -> p t2 x", t2=2)
                        nc.sync.dma_start(y_g[p, 0, :, g, :, :], stA4[:])
                        nc.sync.dma_start(y_g[p, 1, :, g, :, :], stB4[:])
                    elif not fine and g % 2 == 1:
                        # stage layout: stA = [A even chunks; B odd], stB = [B even; A odd]
                        stA4 = stA.rearrange("p (g2 t2 x) -> p g2 t2 x", g2=2, t2=2)
                        stB4 = stB.rearrange("p (g2 t2 x) -> p g2 t2 x", g2=2, t2=2)
                        gsl = slice(g - 1, g + 1)
                        nc.sync.dma_start(y_g[p, 0, :, gsl, :, :], stA4[:])
                        nc.sync.dma_start(y_g[p, 1, :, gsl, :, :], stB4[:])
                    if p == 0:
                        p1_hook(g, cpA, cpB, last_mm)
                conv_scope.__exit__(None, None, None)

    nc.compile()
    return nc


def _host_consts(inputs):
    w_route = np.ascontiguousarray(inputs["w_route"], dtype=np.float32)
    b_route = np.ascontiguousarray(inputs["b_route"], dtype=np.float32)
    w_experts = np.ascontiguousarray(inputs["w_experts"], dtype=np.float32)

    # weT[c, ((e*3+kh)*3+kw)*64 + o] = w_experts[e, o, c, kh, kw]
    wet = w_experts.transpose(2, 0, 3, 4, 1).reshape(C, E * C * 9)
    wet = np.ascontiguousarray(np.concatenate([wet, wet], axis=0))

    consts = np.zeros((128, CC_N), dtype=np.float32)
    consts[0:64, CC_M2COL] = 1.0
    consts[64:128, CC_M2COL + 1] = 1.0
    eye = np.eye(64, dtype=np.float32)
    consts[0:64, CC_STACKI : CC_STACKI + 64] = eye
    consts[64:128, CC_STACKI : CC_STACKI + 64] = eye
    consts[0:C, CC_ROUTE : CC_ROUTE + E] = w_route.T / HW
    consts[C, CC_ROUTE : CC_ROUTE + E] = b_route
    consts[0, CC_MASK2 : CC_MASK2 + 64] = 1.0
    consts[1, CC_MASK2 + 64 : CC_MASK2 + 128] = 1.0
    return wet, consts


def _get_nc():
    if "nc" not in _CACHE:
        _CACHE["nc"] = _build_nc()
    return _CACHE["nc"]


def _run(inputs, trace=False, **kw):
    from concourse import bass_utils

    nc = _get_nc()
    x = np.ascontiguousarray(inputs["x"], dtype=np.float32)
    wet, consts = _host_consts(inputs)
    in_maps = [
        {
            "x": x[i * NS : (i + 1) * NS],
            "weT": wet,
            "consts": consts,
        }
        for i in range(N_CORES)
    ]
    res = bass_utils.run_bass_kernel_spmd(
        nc, in_maps, core_ids=list(range(N_CORES)), trace=trace, **kw
    )
    # reassemble from the diagonal stage layout:
    # y_dev[pp, cls, s, c, g, t2, 4W]; cls0 -> (b,hf)=(s,s), cls1 -> (1-s,s)
    y = np.empty((B, C, H, W), dtype=np.float32)
    yb = y.reshape(N_CORES, NPAIR, 2, C, NT // 2, 2, 2, 4, W)  # b,c,g,t2,hf,r,w
    for i in range(N_CORES):
        yd = np.asarray(res.results[i]["y"]).reshape(
            NPAIR, 2, 2, C, NT // 2, 2, 4, W
        )
        for s in range(2):
            yb[i, :, s, :, :, :, s] = yd[:, 0, s]
            yb[i, :, 1 - s, :, :, :, s] = yd[:, 1, s]
    return y, res


def kernel(**inputs):
    y, _ = _run(inputs)
    return y
